# revision 6
# baseline (speedup 1.0000x reference)
"""Supervised contrastive loss on 8 Trainium2 NeuronCores (Bass/Tile).

Math (matches the reference exactly):
    f_i = x_i / max(||x_i||, 1e-8)                    (normalized rows)
    logits_ij = (f_i . f_j) / T
    mask_ij = [lab_i == lab_j]
    d_i = sum_j exp(logits_ij) * (1 - mask_ij)
    m_i = sum_j mask_ij
    s_i = sum_j mask_ij * (f_i . f_j)                 (raw similarity sum)
    loss = mean_i( log d_i - (s_i / T) / m_i )

Key restructuring so no O(N^2) elementwise mask work is needed:
    - s_i = f_i . t_{lab_i} where t_c = sum_{j: lab_j=c} f_j   (class sums,
      one [100,129] matmul accumulation over row tiles; col 128 = class counts)
    - per-class exp sums via one-hot matmul: W[c, i] = sum_{j in c} exp(G_ji/T),
      computed as OHplus^T @ E with E = exp(G^T/T) tiles [128 j, 512 i].
      Row 100 of OHplus is all-ones -> W[100, i] = total exp sum.
      d_i = W[100, i] - W[lab_i, i].

Sharding: each core computes the [8192, 1024] transposed-similarity slice
against its own row block (separate per-core input => identical SPMD program),
reduces to a single partial scalar; host sums 8 partials.
"""

import numpy as np

import concourse.bass as bass
import concourse.mybir as mybir
import concourse.tile as tile
from concourse import bacc
from concourse.bass_utils import run_bass_kernel_spmd

F32 = mybir.dt.float32
F32R = mybir.dt.float32r
BF16 = mybir.dt.bfloat16
AF = mybir.ActivationFunctionType
OP = mybir.AluOpType

N_CORES = 8
N = 8192
D = 128
C = 100              # label values in [0, 100)
B = N // N_CORES     # 1024 rows per core
NT = N // 128        # 64 full row tiles
BT = B // 128        # 8 own row tiles
MT = B // 512        # 2 moving-dim chunks of the own block
OHW = C + 1          # one-hot width incl. ones column


def _build(inv_t: float, level: int = 4):
    nc = bacc.Bacc("TRN2", target_bir_lowering=False, debug=False,
                   num_devices=N_CORES)

    feat = nc.dram_tensor("feat", [N, D], F32, kind="ExternalInput").ap()
    fown = nc.dram_tensor("fown", [B, D], F32, kind="ExternalInput").ap()
    lab = nc.dram_tensor("lab", [N, 1], F32, kind="ExternalInput").ap()
    labown = nc.dram_tensor("labown", [1, B], F32, kind="ExternalInput").ap()
    iota_bc = nc.dram_tensor("iota_bc", [128, C], F32, kind="ExternalInput").ap()
    iota_pt = nc.dram_tensor("iota_pt", [C, 1], F32, kind="ExternalInput").ap()
    ident = nc.dram_tensor("ident", [128, 128], F32, kind="ExternalInput").ap()
    ones128 = nc.dram_tensor("ones128", [128, 1], F32, kind="ExternalInput").ap()
    partial = nc.dram_tensor("partial", [1, 1], F32, kind="ExternalOutput").ap()

    def _body(tc):
        with tc.tile_pool(name="consts", bufs=1) as consts, \
             tc.tile_pool(name="persist", bufs=1) as persist, \
             tc.tile_pool(name="work", bufs=4) as work, \
             tc.tile_pool(name="epool", bufs=4) as epool:

            # ---- constants ----
            iota_bc_t = consts.tile([128, C], F32, tag="iota_bc")
            iota_pt_t = consts.tile([C, 1], F32, tag="iota_pt")
            ident_t = consts.tile([128, 128], F32, tag="ident")
            ones_t = consts.tile([128, 1], F32, tag="ones")
            labown_t = consts.tile([1, B], F32, tag="labown")
            lab_t = consts.tile([128, NT], F32, tag="lab")
            nc.sync.dma_start(iota_bc_t[:], iota_bc[:])
            nc.sync.dma_start(iota_pt_t[:], iota_pt[:])
            nc.sync.dma_start(ident_t[:], ident[:])
            nc.sync.dma_start(ones_t[:], ones128[:])
            nc.sync.dma_start(labown_t[:], labown[:])
            # lab_t[p, t] = lab[t*128 + p]
            nc.sync.dma_start(
                lab_t[:], lab.rearrange("(t p) o -> p (t o)", p=128))

            # ---- persistent tensors ----
            ft_tiles = [persist.tile([128, 128], F32R, tag=f"ft{i}", name=f"ft{i}")
                        for i in range(NT)]
            fto_tiles = [persist.tile([128, 512], F32R, tag=f"fto{i}", name=f"fto{i}")
                         for i in range(MT)]
            ohp_tiles = [persist.tile([128, C], BF16, tag=f"ohp{i}", name=f"ohp{i}")
                         for i in range(NT)]
            fown_sb = persist.tile([128, BT * (D + 1)], F32, tag="fown_sb")
            tcls_sb = persist.tile([C, D + 1], F32, tag="tcls_sb")
            ohtown = persist.tile([C, B], F32, tag="ohtown")
            notm = persist.tile([C, B], F32, tag="notm")
            labbc = persist.tile([C, B], F32, tag="labbc")

            # OHT_own[c, i] = [lab_own[i] == c]
            nc.gpsimd.partition_broadcast(labbc[:], labown_t[:])
            nc.vector.tensor_scalar(ohtown[:], labbc[:], iota_pt_t[:], None,
                                    OP.is_equal)
            nc.vector.tensor_scalar(notm[:], labbc[:], iota_pt_t[:], None,
                                    OP.not_equal)

            def normalize(dst, src_rows):
                """DMA rows into dst[:, :D], set dst[:, D] = 1, normalize."""
                nc.sync.dma_start(dst[:, 0:D], src_rows)
                nc.vector.memset(dst[:, D:D + 1], 1.0)
                scr = work.tile([128, D], F32, tag="scr")
                ss = work.tile([128, 1], F32, tag="ss")
                nc.scalar.activation(scr[:], dst[:, 0:D], AF.Square,
                                     accum_out=ss[:])
                # scale = max(ss, 1e-16)^-0.5  ==  1/max(sqrt(ss), 1e-8)
                nc.vector.tensor_scalar_max(ss[:], ss[:], 1e-16)
                nc.scalar.activation(ss[:], ss[:], AF.Ln)
                nc.scalar.activation(ss[:], ss[:], AF.Exp, scale=-0.5)
                nc.vector.tensor_scalar_mul(dst[:, 0:D], dst[:, 0:D], ss[:])

            with tc.tile_pool(name="ptrans", bufs=2, space="PSUM") as ptrans, \
                 tc.tile_pool(name="ptcls", bufs=1, space="PSUM") as ptcls:
                tcls_ps = ptcls.tile([C, D + 1], F32, tag="tcls")

                # own block first so the main loop can start early
                for i in range(BT):
                    dst = fown_sb[:, i * (D + 1):(i + 1) * (D + 1)]
                    normalize(dst, fown[i * 128:(i + 1) * 128, :])
                    pt = ptrans.tile([128, 128], F32, tag="pt")
                    nc.tensor.transpose(pt[:], dst[:, 0:D], ident_t[:])
                    nc.vector.tensor_copy(
                        fto_tiles[i // 4][:, (i % 4) * 128:(i % 4 + 1) * 128],
                        pt[:])

                for g in range(NT):
                    xp = work.tile([128, D + 1], F32, tag="xp")
                    normalize(xp, feat[g * 128:(g + 1) * 128, :])
                    fb = work.tile([128, D + 1], BF16, tag="fb")
                    nc.vector.tensor_copy(fb[:], xp[:])
                    oh = ohp_tiles[g]
                    nc.vector.tensor_scalar(oh[:], iota_bc_t[:],
                                            lab_t[:, g:g + 1], None,
                                            OP.is_equal)
                    pt = ptrans.tile([128, 128], F32, tag="pt")
                    nc.tensor.transpose(pt[:], xp[:, 0:D], ident_t[:])
                    nc.vector.tensor_copy(ft_tiles[g][:], pt[:])
                    nc.tensor.matmul(tcls_ps[:], oh[:], fb[:],
                                     start=(g == 0), stop=(g == NT - 1))

                nc.vector.tensor_copy(tcls_sb[:], tcls_ps[:])

            if level == 1:
                res1 = work.tile([1, 1], F32, tag="res1")
                nc.vector.tensor_copy(res1[:], tcls_sb[0:1, 0:1])
                nc.sync.dma_start(partial[:], res1[:])
                return

            with tc.tile_pool(name="pw", bufs=1, space="PSUM") as pw:
                wp = [pw.tile([C, 512], F32, tag=f"w{mt}", name=f"w{mt}")
                      for mt in range(MT)]

                with tc.tile_pool(name="pg", bufs=4, space="PSUM") as pg:
                    for g in range(NT):
                        for mt in range(MT):
                            gp = pg.tile([128, 512], F32, tag="gp")
                            nc.tensor.matmul(
                                gp[:], ft_tiles[g][:],
                                fto_tiles[mt][:], start=True, stop=True)
                            e = epool.tile([128, 512], BF16, tag="e")
                            nc.scalar.activation(e[:], gp[:], AF.Exp,
                                                 scale=inv_t)
                            if level >= 3:
                                nc.tensor.matmul(wp[mt][:], ohp_tiles[g][:],
                                                 e[:], start=(g == 0),
                                                 stop=(g == NT - 1))
                    if level == 2:
                        res2 = work.tile([1, 1], F32, tag="res2")
                        nc.vector.tensor_copy(res2[:], e[0:1, 0:1])
                        nc.sync.dma_start(partial[:], res2[:])
                        return

                if level == 3:
                    res3 = work.tile([1, 1], F32, tag="res3")
                    nc.vector.tensor_copy(res3[:], wp[0][0:1, 0:1])
                    nc.sync.dma_start(partial[:], res3[:])
                    return

                with tc.tile_pool(name="pepi", bufs=2, space="PSUM") as pepi:
                    log_acc = work.tile([1, MT], F32, tag="log_acc")
                    for mt in range(MT):
                        prod = work.tile([C, 512], F32, tag="prod")
                        nc.vector.tensor_tensor(prod[:],
                                                notm[:, mt * 512:(mt + 1) * 512],
                                                wp[mt][:], op=OP.mult)
                        vp = pepi.tile([1, 512], F32, tag="vp")
                        nc.tensor.matmul(vp[:], ones_t[0:C, :], prod[:],
                                         start=True, stop=True)
                        logd = work.tile([1, 512], F32, tag="logd")
                        nc.scalar.activation(logd[:], vp[:], AF.Ln,
                                             accum_out=log_acc[:, mt:mt + 1])

                    qacc = work.tile([128, BT], F32, tag="qacc")
                    for i in range(BT):
                        tsel = pepi.tile([128, D + 1], F32, tag="tsel")
                        nc.tensor.matmul(tsel[:],
                                         ohtown[:, i * 128:(i + 1) * 128],
                                         tcls_sb[0:C, :], start=True, stop=True)
                        scr2 = work.tile([128, D], F32, tag="scr2")
                        s_i = work.tile([128, 1], F32, tag="s_i")
                        nc.vector.tensor_tensor(
                            scr2[:], fown_sb[:, i * (D + 1):i * (D + 1) + D],
                            tsel[:, 0:D], op=OP.mult)
                        nc.vector.tensor_reduce(s_i[:], scr2[:],
                                                axis=mybir.AxisListType.X,
                                                op=OP.add)
                        minv = work.tile([128, 1], F32, tag="minv")
                        nc.vector.reciprocal(minv[:], tsel[:, D:D + 1])
                        nc.vector.tensor_tensor(qacc[:, i:i + 1], s_i[:],
                                                minv[:], op=OP.mult)

                    qsum = work.tile([128, 1], F32, tag="qsum")
                    nc.vector.tensor_reduce(qsum[:], qacc[:],
                                            axis=mybir.AxisListType.X,
                                            op=OP.add)
                    stot = pepi.tile([1, 1], F32, tag="stot")
                    nc.tensor.matmul(stot[:], qsum[:], ones_t[:],
                                     start=True, stop=True)
                    # partial = log_acc[0] + log_acc[1] - inv_t * stot
                    sneg = work.tile([1, 1], F32, tag="sneg")
                    nc.scalar.mul(sneg[:], stot[:], -inv_t)
                    lsum = work.tile([1, 1], F32, tag="lsum")
                    nc.vector.tensor_tensor(lsum[:], log_acc[:, 0:1],
                                            log_acc[:, 1:2], op=OP.add)
                    res = work.tile([1, 1], F32, tag="res")
                    nc.vector.tensor_tensor(res[:], lsum[:], sneg[:],
                                            op=OP.add)
                    nc.sync.dma_start(partial[:], res[:])

    with tile.TileContext(nc) as tc:
        _body(tc)
    nc.compile()
    return nc


_BUILD_CACHE: dict = {}


def _get_nc(inv_t: float):
    if inv_t not in _BUILD_CACHE:
        _BUILD_CACHE[inv_t] = _build(inv_t)
    return _BUILD_CACHE[inv_t]


def kernel(features, labels, temperature):
    features = np.ascontiguousarray(np.asarray(features), dtype=np.float32)
    labels_f = np.asarray(labels).astype(np.float32).reshape(N, 1)
    inv_t = 1.0 / float(temperature)

    nc = _get_nc(inv_t)

    iota_bc = np.broadcast_to(np.arange(C, dtype=np.float32),
                              (128, C)).copy()
    iota_pt = np.arange(C, dtype=np.float32).reshape(C, 1)
    ident = np.eye(128, dtype=np.float32)
    ones128 = np.ones((128, 1), dtype=np.float32)

    in_maps = []
    for c in range(N_CORES):
        sl = slice(c * B, (c + 1) * B)
        in_maps.append({
            "feat": features,
            "fown": features[sl],
            "lab": labels_f,
            "labown": labels_f[sl].reshape(1, B),
            "iota_bc": iota_bc,
            "iota_pt": iota_pt,
            "ident": ident,
            "ones128": ones128,
        })

    res = run_bass_kernel_spmd(nc, in_maps, list(range(N_CORES)))
    total = sum(float(res.results[c]["partial"][0, 0])
                for c in range(N_CORES))
    return np.float32(total / N)
